# revision 1
# baseline (speedup 1.0000x reference)
"""Trainium2 Bass kernel for nn_ControlledConvEMAStabilizer.

Pipeline (per batch image, one NeuronCore each, batch-parallel over 8 cores):
  q = cat(backbone, z, mem_stab, mem_unstab)          # 160ch
  q = lrelu(conv3x3(q, w0) + b0)                      # -> 64ch
  q = lrelu(conv3x3(q, w1) + b1)                      # -> 64ch
  q = lrelu(conv3x3(q, w2) + b2)                      # -> 64ch
  head = conv3x3(q, w_last) + b_last                  # -> 288ch = 9 taps x 32ch
  eta  = softmax([head; 0]) over the 9+1 slots
  out  = sum_p unfold(mem_stab)[p] * eta[p] + eta[9] * z

Implementation notes:
  - Feature maps live in SBUF as zero-padded flat rows: image pixel (r,c) at
    column 129*(r+1)+1+c (row stride 129).  Every 3x3 tap is a pure column
    offset, so convs are PSUM-accumulated matmuls over shifted views.
  - K-stacking: each q tile is [128, NCOL]: partitions 0:64 = q, partitions
    64:128 = q shifted by +129.  A K=128 matmul applies two vertical taps.
  - Strip-PAIR column tiling: the M=64 convs process two 3-row strips
    concurrently as independent PE column-group chains (strip A -> psum[0:64]
    / array cols 0:64, strip B -> psum[64:128] / cols 64:128), ~1.7x faster.
    Strip B's result is written directly to the q tile's UPPER half; one
    mirror DMA per strip completes the K-stack.
  - Inputs are SBUF-resident: xfull [128, NCOL] (backbone+z+mem_stab) and
    mu3 [96, NCOL+2*ST] (3 vertically-shifted copies of mem_unstab; copies 2
    and 3 are SBUF->SBUF shifts of copy 1), loaded in fine column chunks
    spread over both HWDGE queues so conv0 pipelines behind the loads.
  - conv_last: 2x M=128 chunks (serial) + M=32 chunk at col group 2
    (psum[64:96]) concurrent with the PREVIOUS strip's fusion num (col 1) /
    den (col 0) chains.  Final softmax division at base partition 0.
  - DMA descriptors are spread across the two HWDGE queues (sync, scalar)
    plus gpsimd (SWDGE); mem_stab patch loads use 2-strip windows and
    stride-1 tap-group APs (3 taps in one descriptor).
"""

import numpy as np
from contextlib import ExitStack

import concourse.bacc as bacc
import concourse.tile as tile
from concourse import mybir
from concourse.ap import AP
from concourse.bass_utils import run_bass_kernel_spmd

F32 = mybir.dt.float32
BF16 = mybir.dt.bfloat16
ALU = mybir.AluOpType
ACTF = mybir.ActivationFunctionType

H = 128
ST = 129                      # padded row stride
NCOL = ST * 130 + 2           # 16772 sbuf cols
XCOL = NCOL                   # dram padded cols for xpad
MUCOL = NCOL + 2 * ST + 2     # mu dram padded cols (reads up to +258)
RPS = 3                       # rows per strip

# taps in fusion/unfold order p = 3*kh + kw -> offset 129*(kh-1) + (kw-1)
P_TAPS = [ST * (kh - 1) + (kw - 1) for kh in range(3) for kw in range(3)]

W128_OFF = dict(w0c1=0, w1P=576, w2P=768, wlP=960, eye=1824)
W128_COLS = 1856
W96_OFF = dict(w0c2=0, w1S=192, w2S=384, wlS=576)
W96_COLS = 1440


def _j0(r0):
    return ST * (r0 + 1) + 1


def _strips():
    out = []
    r0 = 0
    while r0 < H:
        nr = min(RPS, H - r0)
        out.append((r0, nr))
        r0 += nr
    return out


def _build_program(debug=False):
    nc = bacc.Bacc("TRN2", target_bir_lowering=False, debug=False)

    d_xpad = nc.dram_tensor("xpad", [128, XCOL], BF16, kind="ExternalInput")
    d_mupad = nc.dram_tensor("mupad", [32, MUCOL], BF16, kind="ExternalInput")
    d_w128 = nc.dram_tensor("w128", [128, W128_COLS], BF16, kind="ExternalInput")
    d_w96 = nc.dram_tensor("w96", [96, W96_COLS], BF16, kind="ExternalInput")
    d_b = nc.dram_tensor("bias", [128, 6], F32, kind="ExternalInput")
    d_blp = nc.dram_tensor("blp", [128, 3], F32, kind="ExternalInput")
    d_out = nc.dram_tensor("out", [32, H, H], F32, kind="ExternalOutput")
    if debug:
        d_q1 = nc.dram_tensor("dbg_q1", [128, NCOL], BF16, kind="ExternalOutput")
        d_q2 = nc.dram_tensor("dbg_q2", [128, NCOL], BF16, kind="ExternalOutput")
        d_q3 = nc.dram_tensor("dbg_q3", [128, NCOL], BF16, kind="ExternalOutput")

    strips = _strips()
    pairs = []
    i = 0
    while i < len(strips):
        if i + 1 < len(strips):
            pairs.append((strips[i], strips[i + 1]))
            i += 2
        else:
            pairs.append((strips[i], None))
            i += 1

    with tile.TileContext(nc) as tc, ExitStack() as ctx:
        wp = ctx.enter_context(tc.tile_pool(name="wp", bufs=1))
        big = ctx.enter_context(tc.tile_pool(name="big", bufs=1))
        sm = ctx.enter_context(tc.tile_pool(name="sm", bufs=3))
        fu = ctx.enter_context(tc.tile_pool(name="fu", bufs=2))
        pA = ctx.enter_context(tc.tile_pool(name="pA", bufs=2, space="PSUM"))
        pH = ctx.enter_context(tc.tile_pool(name="pH", bufs=2, space="PSUM"))
        pC3 = ctx.enter_context(tc.tile_pool(name="pC3", bufs=1, space="PSUM"))
        pND = ctx.enter_context(tc.tile_pool(name="pND", bufs=1, space="PSUM"))

        # ---- weights / constants ----
        w128 = wp.tile([128, W128_COLS], BF16)
        w96 = wp.tile([96, W96_COLS], BF16)
        bias = wp.tile([128, 6], F32)
        blp = wp.tile([128, 3], F32)
        nc.sync.dma_start(out=w128[:], in_=d_w128.ap())
        nc.sync.dma_start(out=w96[:], in_=d_w96.ap())
        nc.scalar.dma_start(out=bias[:], in_=d_b.ap())
        nc.scalar.dma_start(out=blp[:], in_=d_blp.ap())

        def w128s(name, i, m0, mw, step=64):
            o = W128_OFF[name] + i * step + m0
            return w128[:, o:o + mw]

        def w96s(name, i, m0, mw, p, step=64):
            o = W96_OFF[name] + i * step + m0
            return w96[0:p, o:o + mw]

        eye = w128[:, W128_OFF["eye"]:W128_OFF["eye"] + 32]

        # ---- SBUF-resident inputs, loaded in fine column chunks in
        # consumption order (alternating HWDGE queues); mu3's two extra
        # vertically-shifted copies are cheap SBUF->SBUF shifts ----
        xfull = wp.tile([128, NCOL], BF16)
        mu3 = wp.tile([96, NCOL + 2 * ST], BF16)
        NX = 12
        xc = [(NCOL * k) // NX for k in range(NX)] + [NCOL]
        MCOL = NCOL + 2 * ST
        mq = [(MCOL * k) // 6 for k in range(6)] + [MCOL]
        for k in range(NX):
            a, b = xc[k], xc[k + 1]
            eng = nc.sync if k % 2 == 0 else nc.scalar
            eng.dma_start(out=xfull[:, a:b], in_=d_xpad.ap()[:, a:b])
            if k < 6:
                a2, b2 = mq[k], mq[k + 1]
                eng2 = nc.scalar if k % 2 == 0 else nc.sync
                eng2.dma_start(out=mu3[0:32, a2:b2], in_=d_mupad.ap()[:, a2:b2])
        for k in range(6):
            a2, b2 = mq[k], min(mq[k + 1], NCOL + ST)
            if a2 < b2:
                nc.scalar.dma_start(out=mu3[32:64, a2:b2],
                                    in_=mu3[0:32, a2 + ST:b2 + ST])
            a3, b3 = mq[k], min(mq[k + 1], NCOL)
            if a3 < b3:
                nc.sync.dma_start(out=mu3[64:96, a3:b3],
                                  in_=mu3[0:32, a3 + 2 * ST:b3 + 2 * ST])

        # ---- q tiles ----
        def new_q(tag):
            q = big.tile([128, NCOL], BF16, tag=tag)
            nc.gpsimd.memset(q[0:64, 0:130], 0.0)
            inter = q[0:64, 258:258 + 127 * ST].rearrange(
                "p (m s) -> p m s", s=ST)[:, :, 0:1]
            nc.gpsimd.memset(inter, 0.0)
            nc.gpsimd.memset(q[0:64, ST * 129:NCOL], 0.0)
            up_inter = q[64:128, 0:ST * 128].rearrange(
                "p (m s) -> p m s", s=ST)[:, :, 0:1]
            nc.gpsimd.memset(up_inter, 0.0)
            last_up = _j0(strips[-1][0]) - ST + strips[-1][1] * ST
            nc.gpsimd.memset(q[64:128, last_up:NCOL], 0.0)
            return q

        def evac_pair(ps, q, pa, pb, bcol):
            (r0a, nra) = pa
            j0a = _j0(r0a)
            na = ST * nra
            rn = sm.tile([128, 3 * ST], F32, tag="rn")
            if pb is not None:
                (r0b, nrb) = pb
                j0b = _j0(r0b)
                nb = ST * nrb
                nmax = max(na, nb)
                nc.scalar.activation(rn[:, 0:nmax], ps[:, 0:nmax], ACTF.Relu,
                                     bias=bias[:, bcol + 1:bcol + 2], scale=-0.99)
            else:
                nc.scalar.activation(rn[0:64, 0:na], ps[0:64, 0:na], ACTF.Relu,
                                     bias=bias[0:64, bcol + 1:bcol + 2], scale=-0.99)
            srcA = ps[0:64, 0:na].rearrange("p (r c) -> p r c", c=ST)[:, :, 0:128]
            rnA = rn[0:64, 0:na].rearrange("p (r c) -> p r c", c=ST)[:, :, 0:128]
            dstA = q[0:64, j0a:j0a + na].rearrange("p (r c) -> p r c", c=ST)[:, :, 0:128]
            nc.vector.scalar_tensor_tensor(dstA, srcA, bias[0:64, bcol:bcol + 1],
                                           rnA, op0=ALU.add, op1=ALU.add)
            nc.sync.dma_start(out=q[64:128, j0a - ST:j0a - ST + na],
                              in_=q[0:64, j0a:j0a + na])
            if pb is None:
                return
            srcB = ps[64:128, 0:nb].rearrange("p (r c) -> p r c", c=ST)[:, :, 0:128]
            rnB = rn[64:128, 0:nb].rearrange("p (r c) -> p r c", c=ST)[:, :, 0:128]
            dstB = q[64:128, j0b - ST:j0b - ST + nb].rearrange(
                "p (r c) -> p r c", c=ST)[:, :, 0:128]
            nc.vector.scalar_tensor_tensor(dstB, srcB, bias[64:128, bcol:bcol + 1],
                                           rnB, op0=ALU.add, op1=ALU.add)
            nc.scalar.dma_start(out=q[0:64, j0b:j0b + nb],
                                in_=q[64:128, j0b - ST:j0b - ST + nb])

        TAPS9 = [(a, b) for a in (-1, 0, 1) for b in (-1, 0, 1)]

        # ================= conv0 (direct reads from xfull / mu3) =============
        q1 = new_q("A")
        for (pa, pb) in pairs:
            j0a = _j0(pa[0])
            na = ST * pa[1]
            j0b = _j0(pb[0]) if pb else 0
            nb = ST * pb[1] if pb else 0
            ps = pA.tile([128, 3 * ST], F32, tag="pA")
            for t in range(12):
                first = (t == 0)
                stop = (t == 11)
                if t < 9:
                    dr, dc = TAPS9[t]
                    oA = j0a + ST * dr + dc
                    oB = j0b + ST * dr + dc
                    nc.tensor.matmul(ps[0:64, 0:na], w128s("w0c1", t, 0, 64),
                                     xfull[:, oA:oA + na], start=first, stop=stop)
                    if pb is not None:
                        nc.tensor.matmul(ps[64:128, 0:nb], w128s("w0c1", t, 0, 64),
                                         xfull[:, oB:oB + nb], start=first, stop=stop)
                else:
                    dc = (-1, 0, 1)[t - 9]
                    oA = j0a - ST + dc
                    oB = j0b - ST + dc
                    nc.tensor.matmul(ps[0:64, 0:na], w96s("w0c2", t - 9, 0, 64, 96),
                                     mu3[0:96, oA:oA + na], start=first, stop=stop)
                    if pb is not None:
                        nc.tensor.matmul(ps[64:128, 0:nb], w96s("w0c2", t - 9, 0, 64, 96),
                                         mu3[0:96, oB:oB + nb], start=first, stop=stop)
            evac_pair(ps, q1, pa, pb, 0)
        if debug:
            nc.sync.dma_start(out=d_q1.ap(), in_=q1[:])

        # ================= conv1 / conv2 =================
        def mid_conv(qin, qout, wPname, wSname, bcol):
            for (pa, pb) in pairs:
                j0a = _j0(pa[0])
                na = ST * pa[1]
                j0b = _j0(pb[0]) if pb else 0
                nb = ST * pb[1] if pb else 0
                ps = pA.tile([128, 3 * ST], F32, tag="pA")
                for t in range(6):
                    first = (t == 0)
                    stop = (t == 5)
                    if t < 3:
                        dc = (-1, 0, 1)[t]
                        oA = j0a - ST + dc
                        oB = j0b - ST + dc
                        nc.tensor.matmul(ps[0:64, 0:na], w128s(wPname, t, 0, 64),
                                         qin[0:128, oA:oA + na], start=first, stop=stop)
                        if pb is not None:
                            nc.tensor.matmul(ps[64:128, 0:nb], w128s(wPname, t, 0, 64),
                                             qin[0:128, oB:oB + nb], start=first, stop=stop)
                    else:
                        dc = (-1, 0, 1)[t - 3]
                        oA = j0a + ST + dc
                        oB = j0b + ST + dc
                        nc.tensor.matmul(ps[0:64, 0:na], w96s(wSname, t - 3, 0, 64, 64),
                                         qin[0:64, oA:oA + na], start=first, stop=stop)
                        if pb is not None:
                            nc.tensor.matmul(ps[64:128, 0:nb], w96s(wSname, t - 3, 0, 64, 64),
                                             qin[0:64, oB:oB + nb], start=first, stop=stop)
                evac_pair(ps, qout, pa, pb, bcol)

        q2 = new_q("B")
        mid_conv(q1, q2, "w1P", "w1S", 2)
        if debug:
            nc.sync.dma_start(out=d_q2.ap(), in_=q2[:])
        q3 = new_q("A")
        mid_conv(q2, q3, "w2P", "w2S", 4)
        if debug:
            nc.sync.dma_start(out=d_q3.ap(), in_=q3[:])

        # ================= conv_last + softmax + fusion =================
        xp_ap = d_xpad.ap()

        def patch_src(tap0, ntap, j0, w):
            # [ntap x 32 x w] from xpad mem_stab rows; taps tap0..tap0+ntap-1
            # have consecutive P_TAPS offsets (stride-1 group)
            return AP(tensor=xp_ap.tensor, offset=96 * XCOL + j0 + P_TAPS[tap0],
                      ap=[[1, ntap], [XCOL, 32], [1, w]])

        def load_pair_patches(s0, npair):
            j0 = _j0(strips[s0][0])
            w = sum(ST * strips[s0 + k][1] for k in range(npair))
            msa = fu.tile([128, 6 * ST + 2], BF16, tag="msa")
            msb = fu.tile([128, 6 * ST + 2], BF16, tag="msb")
            msc = fu.tile([128, 6 * ST + 2], BF16, tag="msc")
            r3p = fu.tile([128, 6 * ST + 2], BF16, tag="r3p")
            nc.gpsimd.dma_start(out=msa[0:96, 0:w], in_=patch_src(0, 3, j0, w))
            nc.sync.dma_start(out=msa[96:128, 0:w], in_=patch_src(3, 1, j0, w))
            nc.gpsimd.dma_start(out=msb[0:64, 0:w], in_=patch_src(4, 2, j0, w))
            nc.gpsimd.dma_start(out=msb[64:128, 0:w], in_=patch_src(6, 2, j0, w))
            nc.sync.dma_start(out=msc[64:96, 0:w], in_=patch_src(8, 1, j0, w))
            nc.sync.dma_start(out=r3p[96:128, 0:w], in_=xp_ap[64:96, j0:j0 + w])
            return msa, msb, msc, r3p

        def chunks12(s):
            (r0, nr) = strips[s]
            j0 = _j0(r0)
            n = ST * nr
            ph0 = pH.tile([128, 3 * ST], F32, tag="pH", name="ph0")
            ph1 = pH.tile([128, 3 * ST], F32, tag="pH", name="ph1")
            for ci, ps in enumerate((ph0, ph1)):
                m0 = 128 * ci
                for i, dc in enumerate((-1, 0, 1)):
                    o = j0 - ST + dc
                    nc.tensor.matmul(ps[:, 0:n], w128s("wlP", i, m0, 128, 288),
                                     q3[0:128, o:o + n], start=(i == 0), stop=False)
                for i, dc in enumerate((-1, 0, 1)):
                    o = j0 + ST + dc
                    nc.tensor.matmul(ps[:, 0:n], w96s("wlS", i, m0, 128, 64, 288),
                                     q3[0:64, o:o + n], start=False, stop=(i == 2))
            return ph0, ph1

        def chunk3_mms(s, pc3):
            # 6 M=32 matmuls -> psum[64:96] at column group 2
            (r0, nr) = strips[s]
            j0 = _j0(r0)
            n = ST * nr
            out = []
            for i, dc in enumerate((-1, 0, 1)):
                o = j0 - ST + dc
                out.append((pc3[64:96, 0:n], w128s("wlP", i, 256, 32, 288),
                            q3[0:128, o:o + n], (i == 0), False, (0, 64)))
            for i, dc in enumerate((-1, 0, 1)):
                o = j0 + ST + dc
                out.append((pc3[64:96, 0:n], w96s("wlS", i, 256, 32, 64, 288),
                            q3[0:64, o:o + n], False, (i == 2), (0, 64)))
            return out

        def numden_mms(st, pnd):
            # num -> pnd[32:64] (col 1), den -> pnd[0:32] (col 0)
            n, off, ta, tb, r3p, ea, eb, ecx = st
            return [
                (pnd[32:64, 0:n], eye, ta[:, 0:n], True, False, (0, 32)),
                (pnd[0:32, 0:n], eye, ea[:, 0:n], True, False, (0, 0)),
                (pnd[32:64, 0:n], eye, tb[:, 0:n], False, False, (0, 32)),
                (pnd[0:32, 0:n], eye, eb[:, 0:n], False, False, (0, 0)),
                (pnd[32:64, 0:n], eye[64:128, :], r3p[64:128, off:off + n],
                 False, True, (64, 32)),
                (pnd[0:32, 0:n], eye[64:96, :], ecx[64:96, 0:n],
                 False, True, (64, 0)),
            ]

        def issue(mms):
            for (out, lhsT, rhs, start, stop, tp) in mms:
                nc.tensor.matmul(out, lhsT, rhs, start=start, stop=stop,
                                 tile_position=tp)

        def fuse_pre(s, off, ph0, ph1, pc3, msa, msb, msc, r3p):
            (r0, nr) = strips[s]
            n = ST * nr
            ea = fu.tile([128, 3 * ST], BF16, tag="ea")
            eb = fu.tile([128, 3 * ST], BF16, tag="eb")
            ecx = fu.tile([128, 3 * ST], BF16, tag="ecx")
            nc.scalar.activation(ea[:, 0:n], ph0[:, 0:n], ACTF.Exp, bias=blp[:, 0:1])
            nc.scalar.activation(eb[:, 0:n], ph1[:, 0:n], ACTF.Exp, bias=blp[:, 1:2])
            nc.scalar.activation(ecx[64:96, 0:n], pc3[64:96, 0:n], ACTF.Exp,
                                 bias=blp[64:96, 2:3])
            ta = fu.tile([128, 3 * ST], BF16, tag="ta")
            tb = fu.tile([128, 3 * ST], BF16, tag="tb")
            nc.vector.tensor_mul(ta[:, 0:n], ea[:, 0:n], msa[:, off:off + n])
            nc.vector.tensor_mul(tb[:, 0:n], eb[:, 0:n], msb[:, off:off + n])
            nc.vector.tensor_mul(r3p[64:96, off:off + n], ecx[64:96, 0:n],
                                 msc[64:96, off:off + n])
            return (n, off, ta, tb, r3p, ea, eb, ecx)

        def fuse_post(s, pnd):
            (r0, nr) = strips[s]
            n = ST * nr
            dn = fu.tile([64, 3 * ST], F32, tag="dn")
            rd = fu.tile([64, 3 * ST], F32, tag="rd")
            ost = fu.tile([64, 3 * ST], F32, tag="ost")
            nc.vector.tensor_scalar_add(dn[0:32, 0:n], pnd[0:32, 0:n], 1.0)
            nc.vector.reciprocal_approx_fast(rd[0:32, 0:n], dn[0:32, 0:n])
            nc.vector.tensor_tensor(ost[0:32, 0:n], pnd[32:64, 0:n],
                                    rd[0:32, 0:n], op=ALU.mult)
            src = ost[0:32, 0:n].rearrange("p (r c) -> p r c", c=ST)[:, :, 0:128]
            nc.sync.dma_start(out=d_out.ap()[:, r0:r0 + nr, :], in_=src)

        prev = None
        for P, (pa, pb) in enumerate(pairs):
            s0 = 2 * P
            npair = 2 if pb is not None else 1
            msa, msb, msc, r3p = load_pair_patches(s0, npair)
            off = 0
            for k in range(npair):
                s = s0 + k
                ph0, ph1 = chunks12(s)
                pc3 = pC3.tile([128, 3 * ST], F32, tag="pC3")
                c3 = chunk3_mms(s, pc3)
                if prev is not None:
                    pst, ppnd, ps_ = prev
                    nd = numden_mms(pst, ppnd)
                    for kk in range(6):
                        issue([c3[kk]])
                        issue([nd[kk]])
                    fuse_post(ps_, ppnd)
                else:
                    issue(c3)
                st = fuse_pre(s, off, ph0, ph1, pc3, msa, msb, msc, r3p)
                pnd = pND.tile([128, 3 * ST], F32, tag="pND")
                prev = (st, pnd, s)
                off += ST * strips[s][1]
        pst, ppnd, ps_ = prev
        issue(numden_mms(pst, ppnd))
        fuse_post(ps_, ppnd)

    nc.compile()
    return nc


BF16_NP = mybir.dt.np(mybir.dt.bfloat16)


def _pad_rows(x, cols):
    c = x.shape[0]
    buf = np.zeros((c, cols), dtype=BF16_NP)
    buf[:, 130:130 + ST * 128].reshape(c, 128, ST)[:, :, 0:128] = x.astype(BF16_NP)
    return buf


def _prep_shared(w0, b0, w1, b1, w2, b2, w_last, b_last):
    f = np.float32
    w0t = np.transpose(np.asarray(w0, f), (1, 2, 3, 0))      # [160,3,3,64]
    w0c1 = np.ascontiguousarray(w0t[0:128].reshape(128, 9 * 64))
    w0c2 = np.ascontiguousarray(
        np.transpose(w0t[128:160], (1, 0, 2, 3)).reshape(96, 3 * 64))
    def mid(w):
        wt = np.transpose(np.asarray(w, f), (1, 2, 3, 0))    # [64,3,3,64]
        wP = np.ascontiguousarray(
            np.concatenate([wt[:, 0], wt[:, 1]], 0).reshape(128, 3 * 64))
        wS = np.ascontiguousarray(wt[:, 2].reshape(64, 3 * 64))
        return wP, wS
    w1P, w1S = mid(w1)
    w2P, w2S = mid(w2)
    perm = np.array([(pp % 32) * 9 + pp // 32 for pp in range(288)])
    wl2 = np.asarray(w_last, f)[perm]                        # [288,64,3,3] p-major
    wlt = np.transpose(wl2, (1, 2, 3, 0))                    # [64,3,3,288]
    wlP = np.ascontiguousarray(
        np.concatenate([wlt[:, 0], wlt[:, 1]], 0).reshape(128, 3 * 288))
    wlS = np.ascontiguousarray(wlt[:, 2].reshape(64, 3 * 288))
    eye = np.tile(np.eye(32, dtype=f), (4, 1))

    w128 = np.zeros((128, W128_COLS), f)
    w128[:, W128_OFF["w0c1"]:W128_OFF["w0c1"] + 576] = w0c1
    w128[:, W128_OFF["w1P"]:W128_OFF["w1P"] + 192] = w1P
    w128[:, W128_OFF["w2P"]:W128_OFF["w2P"] + 192] = w2P
    w128[:, W128_OFF["wlP"]:W128_OFF["wlP"] + 864] = wlP
    w128[:, W128_OFF["eye"]:W128_OFF["eye"] + 32] = eye
    w96 = np.zeros((96, W96_COLS), f)
    w96[0:96, W96_OFF["w0c2"]:W96_OFF["w0c2"] + 192] = w0c2
    w96[0:64, W96_OFF["w1S"]:W96_OFF["w1S"] + 192] = w1S
    w96[0:64, W96_OFF["w2S"]:W96_OFF["w2S"] + 192] = w2S
    w96[0:64, W96_OFF["wlS"]:W96_OFF["wlS"] + 864] = wlS

    b6 = np.stack([np.asarray(b0, f), -0.99 * np.asarray(b0, f),
                   np.asarray(b1, f), -0.99 * np.asarray(b1, f),
                   np.asarray(b2, f), -0.99 * np.asarray(b2, f)], axis=1)
    bias = np.concatenate([b6, b6], axis=0)                  # [128, 6]
    blp_flat = np.asarray(b_last, f)[perm]
    blp = np.zeros((128, 3), f)
    blp[:, 0] = blp_flat[0:128]
    blp[:, 1] = blp_flat[128:256]
    blp[64:96, 2] = blp_flat[256:288]
    out = dict(w128=w128.astype(BF16_NP), w96=w96.astype(BF16_NP),
               bias=np.ascontiguousarray(bias), blp=blp)
    return out


_NC_CACHE = {}


def _get_nc(debug=False):
    if debug not in _NC_CACHE:
        _NC_CACHE[debug] = _build_program(debug)
    return _NC_CACHE[debug]


def make_in_maps(z, backbone, mem_stab, mem_unstab, shared):
    f = np.float32
    z = np.asarray(z, f); backbone = np.asarray(backbone, f)
    ms = np.asarray(mem_stab, f); mu = np.asarray(mem_unstab, f)
    maps = []
    for b in range(z.shape[0]):
        x160 = np.concatenate([backbone[b], z[b], ms[b]], axis=0)  # [128,...]
        maps.append(dict(xpad=_pad_rows(x160, XCOL),
                         mupad=_pad_rows(mu[b], MUCOL), **shared))
    return maps


def kernel(z, backbone, mem_stab, mem_unstab, w0, b0, w1, b1, w2, b2,
           w_last, b_last, fusion_kernel_size):
    assert int(fusion_kernel_size) == 3
    shared = _prep_shared(w0, b0, w1, b1, w2, b2, w_last, b_last)
    in_maps = make_in_maps(z, backbone, mem_stab, mem_unstab, shared)
    nc = _get_nc()
    res = run_bass_kernel_spmd(nc, in_maps, core_ids=list(range(len(in_maps))))
    out = np.stack([r["out"] for r in res.results], axis=0)
    return out.astype(np.float32)



# revision 14
# speedup vs baseline: 1.1548x; 1.1548x over previous
"""Trainium2 Bass kernel for nn_ControlledConvEMAStabilizer.

Pipeline (per batch image, one NeuronCore each, batch-parallel over 8 cores):
  q = cat(backbone, z, mem_stab, mem_unstab)          # 160ch
  q = lrelu(conv3x3(q, w0) + b0)                      # -> 64ch
  q = lrelu(conv3x3(q, w1) + b1)                      # -> 64ch
  q = lrelu(conv3x3(q, w2) + b2)                      # -> 64ch
  head = conv3x3(q, w_last) + b_last                  # -> 288ch = 9 taps x 32ch
  eta  = softmax([head; 0]) over the 9+1 slots
  out  = sum_p unfold(mem_stab)[p] * eta[p] + eta[9] * z

Implementation notes:
  - Feature maps live in SBUF as zero-padded flat rows: image pixel (r,c) at
    column 129*(r+1)+1+c (row stride 129).  Every 3x3 tap is a pure column
    offset, so convs are PSUM-accumulated matmuls over shifted views.
  - K-stacking: each q tile is [128, NCOL]: partitions 0:64 = q, partitions
    64:128 = q shifted by +129.  A K=128 matmul applies two vertical taps.
  - Strip-PAIR column tiling: the M=64 convs process two 3-row strips
    concurrently as independent PE column-group chains (strip A -> psum[0:64]
    / array cols 0:64, strip B -> psum[64:128] / cols 64:128).
  - A dummy warm-up matmul burst runs at t=0 (overlapping the input DMA) to
    lift the PE HAM clock gate (cold 1.2 GHz -> warm 2.4 GHz) before conv0.
  - mu3 (3 vertically-shifted copies of mem_unstab) loads straight from DRAM
    with 3-tap strided APs - no serial SBUF->SBUF shift chain.
  - Conv evac is a single Scalar Lrelu activation (alpha=0.01) writing the
    strided q view directly; one mirror DMA per strip completes the K-stack.
    This keeps Vector free so the PE never stalls on psum evacuation.
  - conv_last + fusion processed in 2-strip blocks:
      * taps 0-7 head -> 2x M=128 psum chunks per strip (serial chains),
      * tap-8 head -> M=32 matmuls col-tiled to PE column groups 2 and 3
        (strip s / s+1) sharing one weight load -> concurrent,
      * softmax reduce: 3 matmul rounds with shared stationary weights
        4-way col-tiled into one psum bank: [den_s, num_s, den_s1, num_s1];
        the custom tail weight folds the softmax "+1" (ones rows) in.
  - Patch loads (mem_stab unfold) use 2-strip windows and stride-1 tap-group
    APs; z and exp/product tails assemble per strip into one [128, n] rhs.
"""

import numpy as np
from contextlib import ExitStack

import concourse.bacc as bacc
import concourse.tile as tile
from concourse import mybir
from concourse.ap import AP
from concourse.bass_utils import run_bass_kernel_spmd

F32 = mybir.dt.float32
BF16 = mybir.dt.bfloat16
ALU = mybir.AluOpType
ACTF = mybir.ActivationFunctionType

H = 128
ST = 129                      # padded row stride
NCOL = ST * 130 + 2           # 16772 sbuf cols
XCOL = NCOL                   # dram padded cols for xpad
MUCOL = NCOL + 2 * ST + 2     # mu dram padded cols (reads up to +258)
RPS = 3                       # rows per strip

# taps in fusion/unfold order p = 3*kh + kw -> offset 129*(kh-1) + (kw-1)
P_TAPS = [ST * (kh - 1) + (kw - 1) for kh in range(3) for kw in range(3)]

W128_OFF = dict(w0c1=0, w1P=576, w2P=768, wlP=960, eye=1824, tail=1856)
W128_COLS = 1920
W96_OFF = dict(w0c2=0, w1S=192, w2S=384, wlS=576)
W96_COLS = 1440


def _j0(r0):
    return ST * (r0 + 1) + 1


def _strips():
    out = []
    r0 = 0
    while r0 < H:
        nr = min(RPS, H - r0)
        out.append((r0, nr))
        r0 += nr
    return out


def _build_program(debug=False):
    nc = bacc.Bacc("TRN2", target_bir_lowering=False, debug=False)

    d_xpad = nc.dram_tensor("xpad", [128, XCOL], BF16, kind="ExternalInput")
    d_mupad = nc.dram_tensor("mupad", [32, MUCOL], BF16, kind="ExternalInput")
    d_w128 = nc.dram_tensor("w128", [128, W128_COLS], BF16, kind="ExternalInput")
    d_w96 = nc.dram_tensor("w96", [96, W96_COLS], BF16, kind="ExternalInput")
    d_b = nc.dram_tensor("bias", [128, 3], F32, kind="ExternalInput")
    d_blp = nc.dram_tensor("blp", [128, 3], F32, kind="ExternalInput")
    d_out = nc.dram_tensor("out", [32, H, H], F32, kind="ExternalOutput")
    if debug:
        d_q1 = nc.dram_tensor("dbg_q1", [128, NCOL], BF16, kind="ExternalOutput")
        d_q2 = nc.dram_tensor("dbg_q2", [128, NCOL], BF16, kind="ExternalOutput")
        d_q3 = nc.dram_tensor("dbg_q3", [128, NCOL], BF16, kind="ExternalOutput")
        d_pc3 = nc.dram_tensor("dbg_pc3", [128, 3 * ST], F32, kind="ExternalOutput")
        d_t80 = nc.dram_tensor("dbg_t80", [128, 3 * ST], BF16, kind="ExternalOutput")
        d_t81 = nc.dram_tensor("dbg_t81", [128, 3 * ST], BF16, kind="ExternalOutput")
        d_pnd = nc.dram_tensor("dbg_pnd", [128, 3 * ST], F32, kind="ExternalOutput")
        d_ea1 = nc.dram_tensor("dbg_ea1", [128, 3 * ST], BF16, kind="ExternalOutput")
        d_ta1 = nc.dram_tensor("dbg_ta1", [128, 3 * ST], BF16, kind="ExternalOutput")
    DBG_BLOCK = 1

    strips = _strips()
    pairs = []
    i = 0
    while i < len(strips):
        if i + 1 < len(strips):
            pairs.append((strips[i], strips[i + 1]))
            i += 2
        else:
            pairs.append((strips[i], None))
            i += 1

    with tile.TileContext(nc) as tc, ExitStack() as ctx:
        wp = ctx.enter_context(tc.tile_pool(name="wp", bufs=1))
        big = ctx.enter_context(tc.tile_pool(name="big", bufs=1))
        fu = ctx.enter_context(tc.tile_pool(name="fu", bufs=2))
        pp = ctx.enter_context(tc.tile_pool(name="pp", bufs=2, space="PSUM"))
        ph = ctx.enter_context(tc.tile_pool(name="ph", bufs=3, space="PSUM"))
        pcp = ctx.enter_context(tc.tile_pool(name="pcp", bufs=2, space="PSUM"))
        pnd = ctx.enter_context(tc.tile_pool(name="pnd", bufs=1, space="PSUM"))

        # ---- HAM warm-up: dense dummy matmul burst, no data deps, runs
        # while the input DMAs stream.  ~18 N=512 matmuls = enough sustained
        # PE busy to lift the clock gate to 2.4 GHz before conv0 begins. ----
        wz = wp.tile([128, 512], BF16)
        nc.gpsimd.memset(wz[:], 0.0)
        wps = pp.tile([128, 512], F32, tag="pA", name="wps")
        for i in range(18):
            nc.tensor.matmul(wps[:, 0:512], wz[:, 0:128], wz[:, 0:512],
                             start=(i == 0), stop=(i == 17))

        # ---- weights / constants ----
        w128 = wp.tile([128, W128_COLS], BF16)
        w96 = wp.tile([96, W96_COLS], BF16)
        bias = wp.tile([128, 3], F32)
        blp = wp.tile([128, 3], F32)
        nc.sync.dma_start(out=w128[:], in_=d_w128.ap())
        nc.scalar.dma_start(out=w96[:], in_=d_w96.ap())
        nc.scalar.dma_start(out=bias[:], in_=d_b.ap())
        nc.scalar.dma_start(out=blp[:], in_=d_blp.ap())

        def w128s(name, i, m0, mw, step=64):
            o = W128_OFF[name] + i * step + m0
            return w128[:, o:o + mw]

        def w96s(name, i, m0, mw, p, step=64):
            o = W96_OFF[name] + i * step + m0
            return w96[0:p, o:o + mw]

        eye = w128[:, W128_OFF["eye"]:W128_OFF["eye"] + 32]
        wtail = w128[:, W128_OFF["tail"]:W128_OFF["tail"] + 64]

        # ---- SBUF-resident inputs, loaded in fine column chunks in
        # consumption order; mu3's 3 vertically-shifted copies come straight
        # from DRAM via 3-tap strided APs (no SBUF->SBUF shift chain). ----
        xfull = wp.tile([128, NCOL], BF16)
        mu3 = wp.tile([96, NCOL], BF16)
        mp_ap = d_mupad.ap()
        NX = 12
        xc = [(NCOL * k) // NX for k in range(NX)] + [NCOL]
        for k in range(NX):
            a, b = xc[k], xc[k + 1]
            eng = nc.sync if k % 2 == 0 else nc.scalar
            eng2 = nc.scalar if k % 2 == 0 else nc.sync
            eng.dma_start(out=xfull[:, a:b], in_=d_xpad.ap()[:, a:b])
            eng2.dma_start(out=mu3[0:96, a:b],
                           in_=AP(tensor=mp_ap.tensor, offset=a,
                                  ap=[[ST, 3], [MUCOL, 32], [1, b - a]]))

        # ---- q tiles ----
        def new_q(tag):
            q = big.tile([128, NCOL], BF16, tag=tag)
            nc.gpsimd.memset(q[0:64, 0:130], 0.0)
            inter = q[0:64, 258:258 + 127 * ST].rearrange(
                "p (m s) -> p m s", s=ST)[:, :, 0:1]
            nc.gpsimd.memset(inter, 0.0)
            nc.gpsimd.memset(q[0:64, ST * 129:NCOL], 0.0)
            up_inter = q[64:128, 0:ST * 128].rearrange(
                "p (m s) -> p m s", s=ST)[:, :, 0:1]
            nc.gpsimd.memset(up_inter, 0.0)
            last_up = _j0(strips[-1][0]) - ST + strips[-1][1] * ST
            nc.gpsimd.memset(q[64:128, last_up:NCOL], 0.0)
            return q

        def view3(ap2d, n):
            return ap2d.rearrange("p (r c) -> p r c", c=ST)[:, :, 0:128]

        def evac_pair(ps, q, pa, pb, bcol):
            # single-op leaky relu: q = lrelu(ps + bias), alpha=0.01
            (r0a, nra) = pa
            j0a = _j0(r0a)
            na = ST * nra
            srcA = view3(ps[0:64, 0:na], na)
            dstA = view3(q[0:64, j0a:j0a + na], na)
            nc.scalar.activation(dstA, srcA, ACTF.Lrelu,
                                 bias=bias[0:64, bcol:bcol + 1], alpha=0.01)
            nc.sync.dma_start(out=q[64:128, j0a - ST:j0a - ST + na],
                              in_=q[0:64, j0a:j0a + na])
            if pb is None:
                return
            (r0b, nrb) = pb
            j0b = _j0(r0b)
            nb = ST * nrb
            srcB = view3(ps[64:128, 0:nb], nb)
            dstB = view3(q[64:128, j0b - ST:j0b - ST + nb], nb)
            nc.scalar.activation(dstB, srcB, ACTF.Lrelu,
                                 bias=bias[64:128, bcol:bcol + 1], alpha=0.01)
            nc.scalar.dma_start(out=q[0:64, j0b:j0b + nb],
                                in_=q[64:128, j0b - ST:j0b - ST + nb])

        TAPS9 = [(a, b) for a in (-1, 0, 1) for b in (-1, 0, 1)]

        # ================= conv0 (direct reads from xfull / mu3) =============
        q1 = new_q("A")
        for (pa, pb) in pairs:
            j0a = _j0(pa[0])
            na = ST * pa[1]
            j0b = _j0(pb[0]) if pb else 0
            nb = ST * pb[1] if pb else 0
            ps = pp.tile([128, 3 * ST], F32, tag="pA")
            for t in range(12):
                first = (t == 0)
                stop = (t == 11)
                if t < 9:
                    dr, dc = TAPS9[t]
                    oA = j0a + ST * dr + dc
                    oB = j0b + ST * dr + dc
                    nc.tensor.matmul(ps[0:64, 0:na], w128s("w0c1", t, 0, 64),
                                     xfull[:, oA:oA + na], start=first, stop=stop)
                    if pb is not None:
                        nc.tensor.matmul(ps[64:128, 0:nb], w128s("w0c1", t, 0, 64),
                                         xfull[:, oB:oB + nb], start=first, stop=stop)
                else:
                    dc = (-1, 0, 1)[t - 9]
                    oA = j0a - ST + dc
                    oB = j0b - ST + dc
                    nc.tensor.matmul(ps[0:64, 0:na], w96s("w0c2", t - 9, 0, 64, 96),
                                     mu3[0:96, oA:oA + na], start=first, stop=stop)
                    if pb is not None:
                        nc.tensor.matmul(ps[64:128, 0:nb], w96s("w0c2", t - 9, 0, 64, 96),
                                         mu3[0:96, oB:oB + nb], start=first, stop=stop)
            evac_pair(ps, q1, pa, pb, 0)
        if debug:
            nc.sync.dma_start(out=d_q1.ap(), in_=q1[:])

        # ================= conv1 / conv2 =================
        def mid_conv(qin, qout, wPname, wSname, bcol):
            for (pa, pb) in pairs:
                j0a = _j0(pa[0])
                na = ST * pa[1]
                j0b = _j0(pb[0]) if pb else 0
                nb = ST * pb[1] if pb else 0
                ps = pp.tile([128, 3 * ST], F32, tag="pA")
                for t in range(6):
                    first = (t == 0)
                    stop = (t == 5)
                    if t < 3:
                        dc = (-1, 0, 1)[t]
                        oA = j0a - ST + dc
                        oB = j0b - ST + dc
                        nc.tensor.matmul(ps[0:64, 0:na], w128s(wPname, t, 0, 64),
                                         qin[0:128, oA:oA + na], start=first, stop=stop)
                        if pb is not None:
                            nc.tensor.matmul(ps[64:128, 0:nb], w128s(wPname, t, 0, 64),
                                             qin[0:128, oB:oB + nb], start=first, stop=stop)
                    else:
                        dc = (-1, 0, 1)[t - 3]
                        oA = j0a + ST + dc
                        oB = j0b + ST + dc
                        nc.tensor.matmul(ps[0:64, 0:na], w96s(wSname, t - 3, 0, 64, 64),
                                         qin[0:64, oA:oA + na], start=first, stop=stop)
                        if pb is not None:
                            nc.tensor.matmul(ps[64:128, 0:nb], w96s(wSname, t - 3, 0, 64, 64),
                                             qin[0:64, oB:oB + nb], start=first, stop=stop)
                evac_pair(ps, qout, pa, pb, bcol)

        q2 = new_q("B")
        mid_conv(q1, q2, "w1P", "w1S", 1)
        if debug:
            nc.sync.dma_start(out=d_q2.ap(), in_=q2[:])
        q3 = new_q("A")
        mid_conv(q2, q3, "w2P", "w2S", 2)
        if debug:
            nc.sync.dma_start(out=d_q3.ap(), in_=q3[:])

        # ================= conv_last + softmax + fusion =================
        xp_ap = d_xpad.ap()

        def patch_src(tap0, ntap, j0, w):
            # [ntap x 32 x w] from xpad mem_stab rows; taps tap0..tap0+ntap-1
            # have consecutive P_TAPS offsets (stride-1 group)
            return AP(tensor=xp_ap.tensor, offset=96 * XCOL + j0 + P_TAPS[tap0],
                      ap=[[1, ntap], [XCOL, 32], [1, w]])

        def head12(s):
            # taps 0-7 of the fusion head for one strip: 2 M=128 psum chunks
            (r0, nr) = strips[s]
            j0 = _j0(r0)
            n = ST * nr
            out = []
            for ci in range(2):
                m0 = 128 * ci
                psc = ph.tile([128, 3 * ST], F32, tag="ph", name=f"ph{ci}")
                for i, dc in enumerate((-1, 0, 1)):
                    o = j0 - ST + dc
                    nc.tensor.matmul(psc[:, 0:n], w128s("wlP", i, m0, 128, 288),
                                     q3[0:128, o:o + n], start=(i == 0), stop=False)
                for i, dc in enumerate((-1, 0, 1)):
                    o = j0 + ST + dc
                    nc.tensor.matmul(psc[:, 0:n], w96s("wlS", i, m0, 128, 64, 288),
                                     q3[0:64, o:o + n], start=False, stop=(i == 2))
                out.append(psc)
            return out

        def head8_pair(s0, s1):
            # tap-8 head for both strips, col-tiled to PE col groups 2 / 3
            # (same stationary weights per tap -> the two matmuls overlap)
            pc3 = pcp.tile([128, 3 * ST], F32, tag="pcp")
            js = [_j0(strips[s0][0]), _j0(strips[s1][0]) if s1 is not None else 0]
            ns = [ST * strips[s0][1], ST * strips[s1][1] if s1 is not None else 0]
            for i, dc in enumerate((-1, 0, 1)):
                for k, (j0, n) in enumerate(zip(js, ns)):
                    if k == 1 and s1 is None:
                        continue
                    o = j0 - ST + dc
                    nc.tensor.matmul(pc3[64 + 32 * k:96 + 32 * k, 0:n],
                                     w128s("wlP", i, 256, 32, 288),
                                     q3[0:128, o:o + n], start=(i == 0), stop=False,
                                     tile_position=(0, 64 + 32 * k))
            for i, dc in enumerate((-1, 0, 1)):
                for k, (j0, n) in enumerate(zip(js, ns)):
                    if k == 1 and s1 is None:
                        continue
                    o = j0 + ST + dc
                    nc.tensor.matmul(pc3[64 + 32 * k:96 + 32 * k, 0:n],
                                     w96s("wlS", i, 256, 32, 64, 288),
                                     q3[0:64, o:o + n], start=False, stop=(i == 2),
                                     tile_position=(0, 64 + 32 * k))
            return pc3

        def numden_mms(st):
            # shared-weight rounds, 4-way col-tiled:
            # pnd = [den_s, den_s1, num_s, num_s1] in one psum bank (dens at
            # base 0 so fuse_post's reciprocal runs once at base 0).
            # tail weights fold the softmax "+1" via the ones rows of t8.
            pndt, parts = st[0], st[1]
            mms = []
            for rnd, key in enumerate(("ta", "tb")):
                for k, pt in enumerate(parts):
                    n = pt["n"]
                    mms.append((pndt[64 + 32 * k:96 + 32 * k, 0:n], eye,
                                pt[key][:, 0:n], rnd == 0, False, (0, 64 + 32 * k)))
                    mms.append((pndt[32 * k:32 + 32 * k, 0:n], eye,
                                pt["e" + key[1]][:, 0:n], rnd == 0, False, (0, 32 * k)))
            for k, pt in enumerate(parts):
                n = pt["n"]
                mms.append((pndt[32 * k:32 + 32 * k, 0:n], wtail[:, 0:32],
                            pt["t8"][:, 0:n], False, True, (0, 32 * k)))
            for k, pt in enumerate(parts):
                n = pt["n"]
                mms.append((pndt[64 + 32 * k:96 + 32 * k, 0:n], wtail[:, 32:64],
                            pt["t8"][:, 0:n], False, True, (0, 64 + 32 * k)))
            return mms

        def issue(mms):
            for (out, lhsT, rhs, start, stop, tp) in mms:
                nc.tensor.matmul(out, lhsT, rhs, start=start, stop=stop,
                                 tile_position=tp, skip_group_check=True)

        def fuse_post(st):
            # all DVE ops same-base (walrus checkSBSameStartPartition); the
            # den->num partition re-alignment goes through a small DMA copy.
            # reciprocal covers both strips' dens in one base-0 op.
            pndt, parts = st[0], st[1]
            np_ = 32 * len(parts)
            n = max(pt["n"] for pt in parts)
            rd = fu.tile([128, 3 * ST], F32, tag="rd", name="rd")
            ost = fu.tile([128, 3 * ST], F32, tag="ost", name="ost")
            nc.vector.reciprocal_approx_fast(rd[0:np_, 0:n], pndt[0:np_, 0:n])
            nc.scalar.dma_start(out=rd[64:64 + np_, 0:n], in_=rd[0:np_, 0:n])
            nc.vector.tensor_tensor(ost[64:64 + np_, 0:n], pndt[64:64 + np_, 0:n],
                                    rd[64:64 + np_, 0:n], op=ALU.mult)
            for k, pt in enumerate(parts):
                (r0, nr) = strips[pt["s"]]
                nst = ST * nr
                src = view3(ost[64 + 32 * k:96 + 32 * k, 0:nst], nst)
                nc.sync.dma_start(out=d_out.ap()[:, r0:r0 + nr, :], in_=src)

        blocks = []
        i = 0
        while i < len(strips):
            if i + 1 < len(strips):
                blocks.append((i, i + 1))
                i += 2
            else:
                blocks.append((i, None))
                i += 1

        def dbg_dump_pnd(st):
            pndt = st[0]
            tmp = fu.tile([128, 3 * ST], F32, tag="dbgtmp", name="dbgtmp")
            nc.vector.tensor_scalar_add(tmp[:, :], pndt[:, :], 0.0)
            nc.sync.dma_start(out=d_pnd.ap(), in_=tmp[:, :])

        prev = None
        for bi, (s0, s1) in enumerate(blocks):
            nstrips = 1 if s1 is None else 2
            j00 = _j0(strips[s0][0])
            w = sum(ST * strips[s0 + k][1] for k in range(nstrips))
            # ---- patch / z loads for the 2-strip window ----
            msa = fu.tile([128, 6 * ST + 2], BF16, tag="msa")
            msb = fu.tile([128, 6 * ST + 2], BF16, tag="msb")
            msc = fu.tile([128, 6 * ST + 2], BF16, tag="msc")
            nc.gpsimd.dma_start(out=msa[0:96, 0:w], in_=patch_src(0, 3, j00, w))
            nc.sync.dma_start(out=msa[96:128, 0:w], in_=patch_src(3, 1, j00, w))
            nc.gpsimd.dma_start(out=msb[0:64, 0:w], in_=patch_src(4, 2, j00, w))
            nc.gpsimd.dma_start(out=msb[64:128, 0:w], in_=patch_src(6, 2, j00, w))
            # tap-8 patch twice, at partition bases 64 and 96, so each strip's
            # product op is same-base with its exp (pc3 col group 2 / 3)
            nc.sync.dma_start(out=msc[64:96, 0:w], in_=patch_src(8, 1, j00, w))
            if nstrips == 2:
                nc.scalar.dma_start(out=msc[96:128, 0:w], in_=patch_src(8, 1, j00, w))
            t8s = []
            for k in range(nstrips):
                s = s0 + k
                j0 = _j0(strips[s][0])
                n = ST * strips[s][1]
                t8 = fu.tile([128, 3 * ST], BF16, tag=f"t8{k}", name="t8")
                eng = nc.sync if k == 0 else nc.scalar
                eng.dma_start(out=t8[32:64, 0:n], in_=xfull[64:96, j0:j0 + n])
                nc.gpsimd.memset(t8[96:128, 0:n], 1.0)
                t8s.append(t8)

            # ---- PE work: previous block's reduce first (its operands are
            # long ready), then this block's head matmuls ----
            if prev is not None:
                issue(numden_mms(prev))
                if debug and prev[2] == DBG_BLOCK:
                    dbg_dump_pnd(prev)
                fuse_post(prev)
            pc3 = head8_pair(s0, s1)
            if debug and bi == DBG_BLOCK:
                tmp3 = fu.tile([128, 3 * ST], F32, tag="dbgtmp3", name="dbgtmp3")
                nc.vector.tensor_scalar_add(tmp3[64:128, :], pc3[64:128, :], 0.0)
                nc.sync.dma_start(out=d_pc3.ap(), in_=tmp3[:, :])
            hp = [head12(s0 + k) for k in range(nstrips)]

            # ---- Scalar exps + Vector products ----
            parts = []
            off = 0
            for k in range(nstrips):
                s = s0 + k
                n = ST * strips[s][1]
                t8 = t8s[k]
                lo, hi = 64 + 32 * k, 96 + 32 * k
                # exp8 / product stay at the pc3 col-group base (same-base DVE),
                # then DMA-assemble into the [prod, z, exp, ones] tail rhs
                e8 = fu.tile([128, 3 * ST], BF16, tag=f"e8{k}", name="e8")
                pr = fu.tile([128, 3 * ST], BF16, tag=f"pr{k}", name="pr")
                nc.scalar.activation(e8[lo:hi, 0:n], pc3[lo:hi, 0:n],
                                     ACTF.Exp, bias=blp[lo:hi, 2:3])
                nc.vector.tensor_mul(pr[lo:hi, 0:n], e8[lo:hi, 0:n],
                                     msc[lo:hi, off:off + n])
                nc.sync.dma_start(out=t8[0:32, 0:n], in_=pr[lo:hi, 0:n])
                nc.scalar.dma_start(out=t8[64:96, 0:n], in_=e8[lo:hi, 0:n])
                ea = fu.tile([128, 3 * ST], BF16, tag=f"ea{k}", name="ea")
                eb = fu.tile([128, 3 * ST], BF16, tag=f"eb{k}", name="eb")
                nc.scalar.activation(ea[:, 0:n], hp[k][0][:, 0:n], ACTF.Exp,
                                     bias=blp[:, 0:1])
                nc.scalar.activation(eb[:, 0:n], hp[k][1][:, 0:n], ACTF.Exp,
                                     bias=blp[:, 1:2])
                ta = fu.tile([128, 3 * ST], BF16, tag=f"ta{k}", name="ta")
                tb = fu.tile([128, 3 * ST], BF16, tag=f"tb{k}", name="tb")
                nc.vector.tensor_mul(ta[:, 0:n], ea[:, 0:n], msa[:, off:off + n])
                nc.vector.tensor_mul(tb[:, 0:n], eb[:, 0:n], msb[:, off:off + n])
                if debug and bi == DBG_BLOCK:
                    nc.sync.dma_start(out=(d_t80 if k == 0 else d_t81).ap(),
                                      in_=t8[:, :])
                    if k == 1:
                        nc.sync.dma_start(out=d_ea1.ap(), in_=ea[:, :])
                        nc.sync.dma_start(out=d_ta1.ap(), in_=ta[:, :])
                parts.append(dict(s=s, n=n, ta=ta, tb=tb, ea=ea, eb=eb, t8=t8))
                off += n
            pndt = pnd.tile([128, 3 * ST], F32, tag="pnd")
            prev = (pndt, parts, bi)

        issue(numden_mms(prev))
        fuse_post(prev)

    nc.compile()
    return nc


BF16_NP = mybir.dt.np(mybir.dt.bfloat16)


def _pad_rows(x, cols):
    c = x.shape[0]
    buf = np.zeros((c, cols), dtype=BF16_NP)
    buf[:, 130:130 + ST * 128].reshape(c, 128, ST)[:, :, 0:128] = x.astype(BF16_NP)
    return buf


def _prep_shared(w0, b0, w1, b1, w2, b2, w_last, b_last):
    f = np.float32
    w0t = np.transpose(np.asarray(w0, f), (1, 2, 3, 0))      # [160,3,3,64]
    w0c1 = np.ascontiguousarray(w0t[0:128].reshape(128, 9 * 64))
    w0c2 = np.ascontiguousarray(
        np.transpose(w0t[128:160], (1, 0, 2, 3)).reshape(96, 3 * 64))
    def mid(w):
        wt = np.transpose(np.asarray(w, f), (1, 2, 3, 0))    # [64,3,3,64]
        wP = np.ascontiguousarray(
            np.concatenate([wt[:, 0], wt[:, 1]], 0).reshape(128, 3 * 64))
        wS = np.ascontiguousarray(wt[:, 2].reshape(64, 3 * 64))
        return wP, wS
    w1P, w1S = mid(w1)
    w2P, w2S = mid(w2)
    perm = np.array([(pp % 32) * 9 + pp // 32 for pp in range(288)])
    wl2 = np.asarray(w_last, f)[perm]                        # [288,64,3,3] p-major
    wlt = np.transpose(wl2, (1, 2, 3, 0))                    # [64,3,3,288]
    wlP = np.ascontiguousarray(
        np.concatenate([wlt[:, 0], wlt[:, 1]], 0).reshape(128, 3 * 288))
    wlS = np.ascontiguousarray(wlt[:, 2].reshape(64, 3 * 288))
    eye = np.tile(np.eye(32, dtype=f), (4, 1))
    i32 = np.eye(32, dtype=f)
    tail = np.zeros((128, 64), f)
    tail[64:96, 0:32] = i32    # den: exp(tap8)
    tail[96:128, 0:32] = i32   # den: +1 (ones rows of t8)
    tail[0:32, 32:64] = i32    # num: exp(tap8)*patch8
    tail[32:64, 32:64] = i32   # num: z (logit-0 slot)

    w128 = np.zeros((128, W128_COLS), f)
    w128[:, W128_OFF["w0c1"]:W128_OFF["w0c1"] + 576] = w0c1
    w128[:, W128_OFF["w1P"]:W128_OFF["w1P"] + 192] = w1P
    w128[:, W128_OFF["w2P"]:W128_OFF["w2P"] + 192] = w2P
    w128[:, W128_OFF["wlP"]:W128_OFF["wlP"] + 864] = wlP
    w128[:, W128_OFF["eye"]:W128_OFF["eye"] + 32] = eye
    w128[:, W128_OFF["tail"]:W128_OFF["tail"] + 64] = tail
    w96 = np.zeros((96, W96_COLS), f)
    w96[0:96, W96_OFF["w0c2"]:W96_OFF["w0c2"] + 192] = w0c2
    w96[0:64, W96_OFF["w1S"]:W96_OFF["w1S"] + 192] = w1S
    w96[0:64, W96_OFF["w2S"]:W96_OFF["w2S"] + 192] = w2S
    w96[0:64, W96_OFF["wlS"]:W96_OFF["wlS"] + 864] = wlS

    b3 = np.stack([np.asarray(b0, f), np.asarray(b1, f), np.asarray(b2, f)],
                  axis=1)                                    # [64, 3]
    bias = np.concatenate([b3, b3], axis=0)                  # [128, 3]
    blp_flat = np.asarray(b_last, f)[perm]
    blp = np.zeros((128, 3), f)
    blp[:, 0] = blp_flat[0:128]
    blp[:, 1] = blp_flat[128:256]
    blp[64:96, 2] = blp_flat[256:288]
    blp[96:128, 2] = blp_flat[256:288]
    out = dict(w128=w128.astype(BF16_NP), w96=w96.astype(BF16_NP),
               bias=np.ascontiguousarray(bias), blp=blp)
    return out


_NC_CACHE = {}


def _get_nc(debug=False):
    if debug not in _NC_CACHE:
        _NC_CACHE[debug] = _build_program(debug)
    return _NC_CACHE[debug]


def make_in_maps(z, backbone, mem_stab, mem_unstab, shared):
    f = np.float32
    z = np.asarray(z, f); backbone = np.asarray(backbone, f)
    ms = np.asarray(mem_stab, f); mu = np.asarray(mem_unstab, f)
    maps = []
    for b in range(z.shape[0]):
        x160 = np.concatenate([backbone[b], z[b], ms[b]], axis=0)  # [128,...]
        maps.append(dict(xpad=_pad_rows(x160, XCOL),
                         mupad=_pad_rows(mu[b], MUCOL), **shared))
    return maps


def kernel(z, backbone, mem_stab, mem_unstab, w0, b0, w1, b1, w2, b2,
           w_last, b_last, fusion_kernel_size):
    assert int(fusion_kernel_size) == 3
    shared = _prep_shared(w0, b0, w1, b1, w2, b2, w_last, b_last)
    in_maps = make_in_maps(z, backbone, mem_stab, mem_unstab, shared)
    nc = _get_nc()
    res = run_bass_kernel_spmd(nc, in_maps, core_ids=list(range(len(in_maps))))
    out = np.stack([r["out"] for r in res.results], axis=0)
    return out.astype(np.float32)


# revision 17
# speedup vs baseline: 1.2848x; 1.1126x over previous
"""Trainium2 Bass kernel for nn_ControlledConvEMAStabilizer.

Pipeline (per batch image, one NeuronCore each, batch-parallel over 8 cores):
  q = cat(backbone, z, mem_stab, mem_unstab)          # 160ch
  q = lrelu(conv3x3(q, w0) + b0)                      # -> 64ch
  q = lrelu(conv3x3(q, w1) + b1)                      # -> 64ch
  q = lrelu(conv3x3(q, w2) + b2)                      # -> 64ch
  head = conv3x3(q, w_last) + b_last                  # -> 288ch = 9 taps x 32ch
  eta  = softmax([head; 0]) over the 9+1 slots
  out  = sum_p unfold(mem_stab)[p] * eta[p] + eta[9] * z

Implementation notes:
  - Feature maps live in SBUF as zero-padded flat rows: image pixel (r,c) at
    column 129*(r+1)+1+c (row stride 129).  Every 3x3 tap is a pure column
    offset, so convs are PSUM-accumulated matmuls over shifted views.
  - K-stacking: each q tile is [128, NCOL]: partitions 0:64 = q, partitions
    64:128 = q shifted by +129.  A K=128 matmul applies two vertical taps.
  - Strip-PAIR column tiling: the M=64 convs process two 3-row strips
    concurrently as independent PE column-group chains (strip A -> psum[0:64]
    / array cols 0:64, strip B -> psum[64:128] / cols 64:128).
  - A dummy warm-up matmul burst runs at t=0 (overlapping the input DMA) to
    lift the PE HAM clock gate (cold 1.2 GHz -> warm 2.4 GHz) before conv0.
  - mu3 (3 vertically-shifted copies of mem_unstab) loads straight from DRAM
    with 3-tap strided APs - no serial SBUF->SBUF shift chain.
  - Conv evac is a single Scalar Lrelu activation (alpha=0.01) writing the
    strided q view directly; one mirror DMA per strip completes the K-stack.
    This keeps Vector free so the PE never stalls on psum evacuation.
  - conv_last + fusion processed in 2-strip blocks:
      * taps 0-7 head -> 2x M=128 psum chunks per strip (serial chains),
      * tap-8 head -> M=32 matmuls col-tiled to PE column groups 2 and 3
        (strip s / s+1) sharing one weight load -> concurrent,
      * softmax reduce: 3 matmul rounds with shared stationary weights
        4-way col-tiled into one psum bank: [den_s, num_s, den_s1, num_s1];
        the custom tail weight folds the softmax "+1" (ones rows) in.
  - Patch loads (mem_stab unfold) use 2-strip windows and stride-1 tap-group
    APs; z and exp/product tails assemble per strip into one [128, n] rhs.
"""

import numpy as np
from contextlib import ExitStack

import concourse.bacc as bacc
import concourse.tile as tile
from concourse import mybir
from concourse.ap import AP
from concourse.bass_utils import run_bass_kernel_spmd

F32 = mybir.dt.float32
BF16 = mybir.dt.bfloat16
ALU = mybir.AluOpType
ACTF = mybir.ActivationFunctionType

H = 128
ST = 129                      # padded row stride
NCOL = ST * 130 + 2           # 16772 sbuf cols
XCOL = NCOL                   # dram padded cols for xpad
MUCOL = NCOL + 2 * ST + 2     # mu dram padded cols (reads up to +258)
RPS = 3                       # rows per strip

# taps in fusion/unfold order p = 3*kh + kw -> offset 129*(kh-1) + (kw-1)
P_TAPS = [ST * (kh - 1) + (kw - 1) for kh in range(3) for kw in range(3)]

W128_OFF = dict(w0c1=0, w1P=576, w2P=768, wlP=960, eye=1824, tail=1856)
W128_COLS = 1920
W96_OFF = dict(w0c2=0, w1S=192, w2S=384, wlS=576)
W96_COLS = 1440


def _j0(r0):
    return ST * (r0 + 1) + 1


def _strips():
    out = []
    r0 = 0
    while r0 < H:
        nr = min(RPS, H - r0)
        out.append((r0, nr))
        r0 += nr
    return out


def _build_program(debug=False):
    nc = bacc.Bacc("TRN2", target_bir_lowering=False, debug=False)

    d_xpad = nc.dram_tensor("xpad", [128, XCOL], BF16, kind="ExternalInput")
    d_mupad = nc.dram_tensor("mupad", [32, MUCOL], BF16, kind="ExternalInput")
    d_w128 = nc.dram_tensor("w128", [128, W128_COLS], BF16, kind="ExternalInput")
    d_w96 = nc.dram_tensor("w96", [96, W96_COLS], BF16, kind="ExternalInput")
    d_b = nc.dram_tensor("bias", [128, 3], F32, kind="ExternalInput")
    d_blp = nc.dram_tensor("blp", [128, 3], F32, kind="ExternalInput")
    d_out = nc.dram_tensor("out", [32, H, H], F32, kind="ExternalOutput")
    if debug:
        d_q1 = nc.dram_tensor("dbg_q1", [128, NCOL], BF16, kind="ExternalOutput")
        d_q2 = nc.dram_tensor("dbg_q2", [128, NCOL], BF16, kind="ExternalOutput")
        d_q3 = nc.dram_tensor("dbg_q3", [128, NCOL], BF16, kind="ExternalOutput")
        d_pc3 = nc.dram_tensor("dbg_pc3", [128, 3 * ST], F32, kind="ExternalOutput")
        d_t80 = nc.dram_tensor("dbg_t80", [128, 3 * ST], BF16, kind="ExternalOutput")
        d_t81 = nc.dram_tensor("dbg_t81", [128, 3 * ST], BF16, kind="ExternalOutput")
        d_pnd = nc.dram_tensor("dbg_pnd", [128, 3 * ST], F32, kind="ExternalOutput")
        d_ea1 = nc.dram_tensor("dbg_ea1", [128, 3 * ST], BF16, kind="ExternalOutput")
        d_ta1 = nc.dram_tensor("dbg_ta1", [128, 3 * ST], BF16, kind="ExternalOutput")
    DBG_BLOCK = 1

    strips = _strips()
    pairs = []
    i = 0
    while i < len(strips):
        if i + 1 < len(strips):
            pairs.append((strips[i], strips[i + 1]))
            i += 2
        else:
            pairs.append((strips[i], None))
            i += 1

    with tile.TileContext(nc) as tc, ExitStack() as ctx:
        wp = ctx.enter_context(tc.tile_pool(name="wp", bufs=1))
        big = ctx.enter_context(tc.tile_pool(name="big", bufs=1))
        fu = ctx.enter_context(tc.tile_pool(name="fu", bufs=2))
        pp = ctx.enter_context(tc.tile_pool(name="pp", bufs=2, space="PSUM"))
        ph = ctx.enter_context(tc.tile_pool(name="ph", bufs=3, space="PSUM"))
        pcp = ctx.enter_context(tc.tile_pool(name="pcp", bufs=2, space="PSUM"))
        pnd = ctx.enter_context(tc.tile_pool(name="pnd", bufs=1, space="PSUM"))

        # ---- HAM warm-up: dense dummy matmul burst, no data deps, runs
        # while the input DMAs stream.  ~18 N=512 matmuls = enough sustained
        # PE busy to lift the clock gate to 2.4 GHz before conv0 begins. ----
        wz = wp.tile([128, 512], BF16)
        nc.gpsimd.memset(wz[:], 0.0)
        wps = pp.tile([128, 512], F32, tag="pA", name="wps")
        for i in range(18):
            nc.tensor.matmul(wps[:, 0:512], wz[:, 0:128], wz[:, 0:512],
                             start=(i == 0), stop=(i == 17))

        # ---- weights / constants ----
        w128 = wp.tile([128, W128_COLS], BF16)
        w96 = wp.tile([96, W96_COLS], BF16)
        bias = wp.tile([128, 3], F32)
        blp = wp.tile([128, 3], F32)
        nc.sync.dma_start(out=w128[:], in_=d_w128.ap())
        nc.scalar.dma_start(out=w96[:], in_=d_w96.ap())
        nc.scalar.dma_start(out=bias[:], in_=d_b.ap())
        nc.scalar.dma_start(out=blp[:], in_=d_blp.ap())

        def w128s(name, i, m0, mw, step=64):
            o = W128_OFF[name] + i * step + m0
            return w128[:, o:o + mw]

        def w96s(name, i, m0, mw, p, step=64):
            o = W96_OFF[name] + i * step + m0
            return w96[0:p, o:o + mw]

        eye = w128[:, W128_OFF["eye"]:W128_OFF["eye"] + 32]
        wtail = w128[:, W128_OFF["tail"]:W128_OFF["tail"] + 64]

        # ---- SBUF-resident inputs, loaded in fine column chunks in
        # consumption order; mu3's 3 vertically-shifted copies come straight
        # from DRAM via 3-tap strided APs (no SBUF->SBUF shift chain). ----
        xfull = wp.tile([128, NCOL], BF16)
        mu3 = wp.tile([96, NCOL], BF16)
        mp_ap = d_mupad.ap()
        # geometric chunks: small leading chunks unblock conv0's first pairs
        # quickly, large tail chunks amortize; xfull on sync/scalar queues,
        # mu3 on vector/gpsimd queues -> 4-5 queues stream in parallel
        xc = [0, 1000, 2000, 3200, 4400, 5800, 7400, 9400, 11800, 14200, NCOL]
        for k in range(len(xc) - 1):
            a, b = xc[k], xc[k + 1]
            eng = nc.sync if k % 2 == 0 else nc.scalar
            eng2 = nc.gpsimd
            eng.dma_start(out=xfull[:, a:b], in_=d_xpad.ap()[:, a:b])
            eng2.dma_start(out=mu3[0:96, a:b],
                           in_=AP(tensor=mp_ap.tensor, offset=a,
                                  ap=[[ST, 3], [MUCOL, 32], [1, b - a]]))

        # ---- q tiles ----
        def new_q(tag):
            q = big.tile([128, NCOL], BF16, tag=tag)
            nc.gpsimd.memset(q[0:64, 0:130], 0.0)
            inter = q[0:64, 258:258 + 127 * ST].rearrange(
                "p (m s) -> p m s", s=ST)[:, :, 0:1]
            nc.gpsimd.memset(inter, 0.0)
            nc.gpsimd.memset(q[0:64, ST * 129:NCOL], 0.0)
            up_inter = q[64:128, 0:ST * 128].rearrange(
                "p (m s) -> p m s", s=ST)[:, :, 0:1]
            nc.gpsimd.memset(up_inter, 0.0)
            last_up = _j0(strips[-1][0]) - ST + strips[-1][1] * ST
            nc.gpsimd.memset(q[64:128, last_up:NCOL], 0.0)
            return q

        def view3(ap2d, n):
            return ap2d.rearrange("p (r c) -> p r c", c=ST)[:, :, 0:128]

        def evac_pair(ps, q, pa, pb, bcol):
            # single-op leaky relu: q = lrelu(ps + bias), alpha=0.01
            (r0a, nra) = pa
            j0a = _j0(r0a)
            na = ST * nra
            srcA = view3(ps[0:64, 0:na], na)
            dstA = view3(q[0:64, j0a:j0a + na], na)
            nc.scalar.activation(dstA, srcA, ACTF.Lrelu,
                                 bias=bias[0:64, bcol:bcol + 1], alpha=0.01)
            nc.sync.dma_start(out=q[64:128, j0a - ST:j0a - ST + na],
                              in_=q[0:64, j0a:j0a + na])
            if pb is None:
                return
            (r0b, nrb) = pb
            j0b = _j0(r0b)
            nb = ST * nrb
            srcB = view3(ps[64:128, 0:nb], nb)
            dstB = view3(q[64:128, j0b - ST:j0b - ST + nb], nb)
            nc.scalar.activation(dstB, srcB, ACTF.Lrelu,
                                 bias=bias[64:128, bcol:bcol + 1], alpha=0.01)
            nc.scalar.dma_start(out=q[0:64, j0b:j0b + nb],
                                in_=q[64:128, j0b - ST:j0b - ST + nb])

        TAPS9 = [(a, b) for a in (-1, 0, 1) for b in (-1, 0, 1)]

        # ================= conv0 (direct reads from xfull / mu3) =============
        q1 = new_q("A")
        for (pa, pb) in pairs:
            j0a = _j0(pa[0])
            na = ST * pa[1]
            j0b = _j0(pb[0]) if pb else 0
            nb = ST * pb[1] if pb else 0
            ps = pp.tile([128, 3 * ST], F32, tag="pA")
            for t in range(12):
                first = (t == 0)
                stop = (t == 11)
                if t < 9:
                    dr, dc = TAPS9[t]
                    oA = j0a + ST * dr + dc
                    oB = j0b + ST * dr + dc
                    nc.tensor.matmul(ps[0:64, 0:na], w128s("w0c1", t, 0, 64),
                                     xfull[:, oA:oA + na], start=first, stop=stop)
                    if pb is not None:
                        nc.tensor.matmul(ps[64:128, 0:nb], w128s("w0c1", t, 0, 64),
                                         xfull[:, oB:oB + nb], start=first, stop=stop)
                else:
                    dc = (-1, 0, 1)[t - 9]
                    oA = j0a - ST + dc
                    oB = j0b - ST + dc
                    nc.tensor.matmul(ps[0:64, 0:na], w96s("w0c2", t - 9, 0, 64, 96),
                                     mu3[0:96, oA:oA + na], start=first, stop=stop)
                    if pb is not None:
                        nc.tensor.matmul(ps[64:128, 0:nb], w96s("w0c2", t - 9, 0, 64, 96),
                                         mu3[0:96, oB:oB + nb], start=first, stop=stop)
            evac_pair(ps, q1, pa, pb, 0)
        if debug:
            nc.sync.dma_start(out=d_q1.ap(), in_=q1[:])

        # ================= conv1 / conv2 =================
        def mid_conv(qin, qout, wPname, wSname, bcol):
            for (pa, pb) in pairs:
                j0a = _j0(pa[0])
                na = ST * pa[1]
                j0b = _j0(pb[0]) if pb else 0
                nb = ST * pb[1] if pb else 0
                ps = pp.tile([128, 3 * ST], F32, tag="pA")
                for t in range(6):
                    first = (t == 0)
                    stop = (t == 5)
                    if t < 3:
                        dc = (-1, 0, 1)[t]
                        oA = j0a - ST + dc
                        oB = j0b - ST + dc
                        nc.tensor.matmul(ps[0:64, 0:na], w128s(wPname, t, 0, 64),
                                         qin[0:128, oA:oA + na], start=first, stop=stop)
                        if pb is not None:
                            nc.tensor.matmul(ps[64:128, 0:nb], w128s(wPname, t, 0, 64),
                                             qin[0:128, oB:oB + nb], start=first, stop=stop)
                    else:
                        dc = (-1, 0, 1)[t - 3]
                        oA = j0a + ST + dc
                        oB = j0b + ST + dc
                        nc.tensor.matmul(ps[0:64, 0:na], w96s(wSname, t - 3, 0, 64, 64),
                                         qin[0:64, oA:oA + na], start=first, stop=stop)
                        if pb is not None:
                            nc.tensor.matmul(ps[64:128, 0:nb], w96s(wSname, t - 3, 0, 64, 64),
                                             qin[0:64, oB:oB + nb], start=first, stop=stop)
                evac_pair(ps, qout, pa, pb, bcol)

        q2 = new_q("B")
        mid_conv(q1, q2, "w1P", "w1S", 1)
        if debug:
            nc.sync.dma_start(out=d_q2.ap(), in_=q2[:])
        q3 = new_q("A")
        mid_conv(q2, q3, "w2P", "w2S", 2)
        if debug:
            nc.sync.dma_start(out=d_q3.ap(), in_=q3[:])

        # ================= conv_last + softmax + fusion =================
        xp_ap = d_xpad.ap()

        def patch_src(tap0, ntap, j0, w):
            # [ntap x 32 x w] from xpad mem_stab rows; taps tap0..tap0+ntap-1
            # have consecutive P_TAPS offsets (stride-1 group)
            return AP(tensor=xp_ap.tensor, offset=96 * XCOL + j0 + P_TAPS[tap0],
                      ap=[[1, ntap], [XCOL, 32], [1, w]])

        def head12(s):
            # taps 0-7 of the fusion head for one strip: 2 M=128 psum chunks
            (r0, nr) = strips[s]
            j0 = _j0(r0)
            n = ST * nr
            out = []
            for ci in range(2):
                m0 = 128 * ci
                psc = ph.tile([128, 3 * ST], F32, tag="ph", name=f"ph{ci}")
                for i, dc in enumerate((-1, 0, 1)):
                    o = j0 - ST + dc
                    nc.tensor.matmul(psc[:, 0:n], w128s("wlP", i, m0, 128, 288),
                                     q3[0:128, o:o + n], start=(i == 0), stop=False)
                for i, dc in enumerate((-1, 0, 1)):
                    o = j0 + ST + dc
                    nc.tensor.matmul(psc[:, 0:n], w96s("wlS", i, m0, 128, 64, 288),
                                     q3[0:64, o:o + n], start=False, stop=(i == 2))
                out.append(psc)
            return out

        def head8_pair(s0, s1):
            # tap-8 head for both strips, col-tiled to PE col groups 2 / 3
            # (same stationary weights per tap -> the two matmuls overlap)
            pc3 = pcp.tile([128, 3 * ST], F32, tag="pcp")
            js = [_j0(strips[s0][0]), _j0(strips[s1][0]) if s1 is not None else 0]
            ns = [ST * strips[s0][1], ST * strips[s1][1] if s1 is not None else 0]
            for i, dc in enumerate((-1, 0, 1)):
                for k, (j0, n) in enumerate(zip(js, ns)):
                    if k == 1 and s1 is None:
                        continue
                    o = j0 - ST + dc
                    nc.tensor.matmul(pc3[64 + 32 * k:96 + 32 * k, 0:n],
                                     w128s("wlP", i, 256, 32, 288),
                                     q3[0:128, o:o + n], start=(i == 0), stop=False,
                                     tile_position=(0, 64 + 32 * k))
            for i, dc in enumerate((-1, 0, 1)):
                for k, (j0, n) in enumerate(zip(js, ns)):
                    if k == 1 and s1 is None:
                        continue
                    o = j0 + ST + dc
                    nc.tensor.matmul(pc3[64 + 32 * k:96 + 32 * k, 0:n],
                                     w96s("wlS", i, 256, 32, 64, 288),
                                     q3[0:64, o:o + n], start=False, stop=(i == 2),
                                     tile_position=(0, 64 + 32 * k))
            return pc3

        def numden_mms(st):
            # shared-weight rounds, 4-way col-tiled:
            # pnd = [den_s, den_s1, num_s, num_s1] in one psum bank (dens at
            # base 0 so fuse_post's reciprocal runs once at base 0).
            # tail weights fold the softmax "+1" via the ones rows of t8.
            pndt, parts = st[0], st[1]
            mms = []
            for rnd, key in enumerate(("ta", "tb")):
                for k, pt in enumerate(parts):
                    n = pt["n"]
                    mms.append((pndt[64 + 32 * k:96 + 32 * k, 0:n], eye,
                                pt[key][:, 0:n], rnd == 0, False, (0, 64 + 32 * k)))
                    mms.append((pndt[32 * k:32 + 32 * k, 0:n], eye,
                                pt["e" + key[1]][:, 0:n], rnd == 0, False, (0, 32 * k)))
            for k, pt in enumerate(parts):
                n = pt["n"]
                mms.append((pndt[32 * k:32 + 32 * k, 0:n], wtail[:, 0:32],
                            pt["t8"][:, 0:n], False, True, (0, 32 * k)))
            for k, pt in enumerate(parts):
                n = pt["n"]
                mms.append((pndt[64 + 32 * k:96 + 32 * k, 0:n], wtail[:, 32:64],
                            pt["t8"][:, 0:n], False, True, (0, 64 + 32 * k)))
            return mms

        def issue(mms):
            for (out, lhsT, rhs, start, stop, tp) in mms:
                nc.tensor.matmul(out, lhsT, rhs, start=start, stop=stop,
                                 tile_position=tp, skip_group_check=True)

        def fuse_post(st):
            # all DVE ops same-base (walrus checkSBSameStartPartition); the
            # den->num partition re-alignment goes through a small DMA copy.
            # reciprocal covers both strips' dens in one base-0 op.
            pndt, parts = st[0], st[1]
            np_ = 32 * len(parts)
            n = max(pt["n"] for pt in parts)
            rd = fu.tile([128, 3 * ST], F32, tag="rd", name="rd")
            ost = fu.tile([128, 3 * ST], F32, tag="ost", name="ost")
            nc.vector.reciprocal_approx_fast(rd[0:np_, 0:n], pndt[0:np_, 0:n])
            nc.scalar.dma_start(out=rd[64:64 + np_, 0:n], in_=rd[0:np_, 0:n])
            nc.vector.tensor_tensor(ost[64:64 + np_, 0:n], pndt[64:64 + np_, 0:n],
                                    rd[64:64 + np_, 0:n], op=ALU.mult)
            for k, pt in enumerate(parts):
                (r0, nr) = strips[pt["s"]]
                nst = ST * nr
                src = view3(ost[64 + 32 * k:96 + 32 * k, 0:nst], nst)
                nc.sync.dma_start(out=d_out.ap()[:, r0:r0 + nr, :], in_=src)

        blocks = []
        i = 0
        while i < len(strips):
            if i + 1 < len(strips):
                blocks.append((i, i + 1))
                i += 2
            else:
                blocks.append((i, None))
                i += 1

        def dbg_dump_pnd(st):
            pndt = st[0]
            tmp = fu.tile([128, 3 * ST], F32, tag="dbgtmp", name="dbgtmp")
            nc.vector.tensor_scalar_add(tmp[:, :], pndt[:, :], 0.0)
            nc.sync.dma_start(out=d_pnd.ap(), in_=tmp[:, :])

        prev = None
        for bi, (s0, s1) in enumerate(blocks):
            nstrips = 1 if s1 is None else 2
            j00 = _j0(strips[s0][0])
            w = sum(ST * strips[s0 + k][1] for k in range(nstrips))
            # ---- patch / z loads for the 2-strip window ----
            msa = fu.tile([128, 6 * ST + 2], BF16, tag="msa")
            msb = fu.tile([128, 6 * ST + 2], BF16, tag="msb")
            msc = fu.tile([128, 6 * ST + 2], BF16, tag="msc")
            nc.gpsimd.dma_start(out=msa[0:96, 0:w], in_=patch_src(0, 3, j00, w))
            nc.sync.dma_start(out=msa[96:128, 0:w], in_=patch_src(3, 1, j00, w))
            nc.gpsimd.dma_start(out=msb[0:64, 0:w], in_=patch_src(4, 2, j00, w))
            nc.gpsimd.dma_start(out=msb[64:128, 0:w], in_=patch_src(6, 2, j00, w))
            # tap-8 patch twice, at partition bases 64 and 96, so each strip's
            # product op is same-base with its exp (pc3 col group 2 / 3)
            nc.sync.dma_start(out=msc[64:96, 0:w], in_=patch_src(8, 1, j00, w))
            if nstrips == 2:
                nc.scalar.dma_start(out=msc[96:128, 0:w], in_=patch_src(8, 1, j00, w))
            t8s = []
            for k in range(nstrips):
                s = s0 + k
                j0 = _j0(strips[s][0])
                n = ST * strips[s][1]
                t8 = fu.tile([128, 3 * ST], BF16, tag=f"t8{k}", name="t8")
                eng = nc.sync if k == 0 else nc.scalar
                eng.dma_start(out=t8[32:64, 0:n], in_=xfull[64:96, j0:j0 + n])
                nc.gpsimd.memset(t8[96:128, 0:n], 1.0)
                t8s.append(t8)

            # ---- PE work: previous block's reduce first (its operands are
            # long ready), then this block's head matmuls ----
            # head matmuls first; the PREVIOUS block's softmax reduce is
            # issued after them so its operand chain (exp -> mul -> t8 DMA
            # assembly) has a full block of slack -> no PE stall at block
            # boundaries (a >1us PE idle re-throttles the HAM clock gate)
            pc3 = head8_pair(s0, s1)
            if debug and bi == DBG_BLOCK:
                tmp3 = fu.tile([128, 3 * ST], F32, tag="dbgtmp3", name="dbgtmp3")
                nc.vector.tensor_scalar_add(tmp3[64:128, :], pc3[64:128, :], 0.0)
                nc.sync.dma_start(out=d_pc3.ap(), in_=tmp3[:, :])
            hp = [head12(s0 + k) for k in range(nstrips)]
            if prev is not None:
                issue(numden_mms(prev))
                if debug and prev[2] == DBG_BLOCK:
                    dbg_dump_pnd(prev)
                fuse_post(prev)

            # ---- Scalar exps + Vector products ----
            parts = []
            off = 0
            for k in range(nstrips):
                s = s0 + k
                n = ST * strips[s][1]
                t8 = t8s[k]
                lo, hi = 64 + 32 * k, 96 + 32 * k
                # exp8 / product stay at the pc3 col-group base (same-base DVE),
                # then DMA-assemble into the [prod, z, exp, ones] tail rhs
                e8 = fu.tile([128, 3 * ST], BF16, tag=f"e8{k}", name="e8")
                pr = fu.tile([128, 3 * ST], BF16, tag=f"pr{k}", name="pr")
                nc.scalar.activation(e8[lo:hi, 0:n], pc3[lo:hi, 0:n],
                                     ACTF.Exp, bias=blp[lo:hi, 2:3])
                nc.vector.tensor_mul(pr[lo:hi, 0:n], e8[lo:hi, 0:n],
                                     msc[lo:hi, off:off + n])
                nc.sync.dma_start(out=t8[0:32, 0:n], in_=pr[lo:hi, 0:n])
                nc.scalar.dma_start(out=t8[64:96, 0:n], in_=e8[lo:hi, 0:n])
                ea = fu.tile([128, 3 * ST], BF16, tag=f"ea{k}", name="ea")
                eb = fu.tile([128, 3 * ST], BF16, tag=f"eb{k}", name="eb")
                nc.scalar.activation(ea[:, 0:n], hp[k][0][:, 0:n], ACTF.Exp,
                                     bias=blp[:, 0:1])
                nc.scalar.activation(eb[:, 0:n], hp[k][1][:, 0:n], ACTF.Exp,
                                     bias=blp[:, 1:2])
                ta = fu.tile([128, 3 * ST], BF16, tag=f"ta{k}", name="ta")
                tb = fu.tile([128, 3 * ST], BF16, tag=f"tb{k}", name="tb")
                nc.vector.tensor_mul(ta[:, 0:n], ea[:, 0:n], msa[:, off:off + n])
                nc.vector.tensor_mul(tb[:, 0:n], eb[:, 0:n], msb[:, off:off + n])
                if debug and bi == DBG_BLOCK:
                    nc.sync.dma_start(out=(d_t80 if k == 0 else d_t81).ap(),
                                      in_=t8[:, :])
                    if k == 1:
                        nc.sync.dma_start(out=d_ea1.ap(), in_=ea[:, :])
                        nc.sync.dma_start(out=d_ta1.ap(), in_=ta[:, :])
                parts.append(dict(s=s, n=n, ta=ta, tb=tb, ea=ea, eb=eb, t8=t8))
                off += n
            pndt = pnd.tile([128, 3 * ST], F32, tag="pnd")
            prev = (pndt, parts, bi)

        issue(numden_mms(prev))
        fuse_post(prev)

    nc.compile()
    return nc


BF16_NP = mybir.dt.np(mybir.dt.bfloat16)


def _pad_rows(x, cols):
    c = x.shape[0]
    buf = np.zeros((c, cols), dtype=BF16_NP)
    buf[:, 130:130 + ST * 128].reshape(c, 128, ST)[:, :, 0:128] = x.astype(BF16_NP)
    return buf


def _prep_shared(w0, b0, w1, b1, w2, b2, w_last, b_last):
    f = np.float32
    w0t = np.transpose(np.asarray(w0, f), (1, 2, 3, 0))      # [160,3,3,64]
    w0c1 = np.ascontiguousarray(w0t[0:128].reshape(128, 9 * 64))
    w0c2 = np.ascontiguousarray(
        np.transpose(w0t[128:160], (1, 0, 2, 3)).reshape(96, 3 * 64))
    def mid(w):
        wt = np.transpose(np.asarray(w, f), (1, 2, 3, 0))    # [64,3,3,64]
        wP = np.ascontiguousarray(
            np.concatenate([wt[:, 0], wt[:, 1]], 0).reshape(128, 3 * 64))
        wS = np.ascontiguousarray(wt[:, 2].reshape(64, 3 * 64))
        return wP, wS
    w1P, w1S = mid(w1)
    w2P, w2S = mid(w2)
    perm = np.array([(pp % 32) * 9 + pp // 32 for pp in range(288)])
    wl2 = np.asarray(w_last, f)[perm]                        # [288,64,3,3] p-major
    wlt = np.transpose(wl2, (1, 2, 3, 0))                    # [64,3,3,288]
    wlP = np.ascontiguousarray(
        np.concatenate([wlt[:, 0], wlt[:, 1]], 0).reshape(128, 3 * 288))
    wlS = np.ascontiguousarray(wlt[:, 2].reshape(64, 3 * 288))
    eye = np.tile(np.eye(32, dtype=f), (4, 1))
    i32 = np.eye(32, dtype=f)
    tail = np.zeros((128, 64), f)
    tail[64:96, 0:32] = i32    # den: exp(tap8)
    tail[96:128, 0:32] = i32   # den: +1 (ones rows of t8)
    tail[0:32, 32:64] = i32    # num: exp(tap8)*patch8
    tail[32:64, 32:64] = i32   # num: z (logit-0 slot)

    w128 = np.zeros((128, W128_COLS), f)
    w128[:, W128_OFF["w0c1"]:W128_OFF["w0c1"] + 576] = w0c1
    w128[:, W128_OFF["w1P"]:W128_OFF["w1P"] + 192] = w1P
    w128[:, W128_OFF["w2P"]:W128_OFF["w2P"] + 192] = w2P
    w128[:, W128_OFF["wlP"]:W128_OFF["wlP"] + 864] = wlP
    w128[:, W128_OFF["eye"]:W128_OFF["eye"] + 32] = eye
    w128[:, W128_OFF["tail"]:W128_OFF["tail"] + 64] = tail
    w96 = np.zeros((96, W96_COLS), f)
    w96[0:96, W96_OFF["w0c2"]:W96_OFF["w0c2"] + 192] = w0c2
    w96[0:64, W96_OFF["w1S"]:W96_OFF["w1S"] + 192] = w1S
    w96[0:64, W96_OFF["w2S"]:W96_OFF["w2S"] + 192] = w2S
    w96[0:64, W96_OFF["wlS"]:W96_OFF["wlS"] + 864] = wlS

    b3 = np.stack([np.asarray(b0, f), np.asarray(b1, f), np.asarray(b2, f)],
                  axis=1)                                    # [64, 3]
    bias = np.concatenate([b3, b3], axis=0)                  # [128, 3]
    blp_flat = np.asarray(b_last, f)[perm]
    blp = np.zeros((128, 3), f)
    blp[:, 0] = blp_flat[0:128]
    blp[:, 1] = blp_flat[128:256]
    blp[64:96, 2] = blp_flat[256:288]
    blp[96:128, 2] = blp_flat[256:288]
    out = dict(w128=w128.astype(BF16_NP), w96=w96.astype(BF16_NP),
               bias=np.ascontiguousarray(bias), blp=blp)
    return out


_NC_CACHE = {}


def _get_nc(debug=False):
    if debug not in _NC_CACHE:
        _NC_CACHE[debug] = _build_program(debug)
    return _NC_CACHE[debug]


def make_in_maps(z, backbone, mem_stab, mem_unstab, shared):
    f = np.float32
    z = np.asarray(z, f); backbone = np.asarray(backbone, f)
    ms = np.asarray(mem_stab, f); mu = np.asarray(mem_unstab, f)
    maps = []
    for b in range(z.shape[0]):
        x160 = np.concatenate([backbone[b], z[b], ms[b]], axis=0)  # [128,...]
        maps.append(dict(xpad=_pad_rows(x160, XCOL),
                         mupad=_pad_rows(mu[b], MUCOL), **shared))
    return maps


def kernel(z, backbone, mem_stab, mem_unstab, w0, b0, w1, b1, w2, b2,
           w_last, b_last, fusion_kernel_size):
    assert int(fusion_kernel_size) == 3
    shared = _prep_shared(w0, b0, w1, b1, w2, b2, w_last, b_last)
    in_maps = make_in_maps(z, backbone, mem_stab, mem_unstab, shared)
    nc = _get_nc()
    res = run_bass_kernel_spmd(nc, in_maps, core_ids=list(range(len(in_maps))))
    out = np.stack([r["out"] for r in res.results], axis=0)
    return out.astype(np.float32)
